# revision 9
# baseline (speedup 1.0000x reference)
"""BandSplit (BSRNN-style) Trainium2 kernel — f-window streaming design.

Folded form (r_b = rsqrt(var+eps); mu, r_b per band+sample):
  y = r_b * (Wg @ h) + (v + b_band - r_b*mu*u)
  Wg = W * gamma (per column), u = sum_cols(Wg_bf16), v = W @ beta + b_band
so the big matmul runs on raw h and GroupNorm is a per-band scalar scale plus
a per-output-channel bias on the matmul output.

Data layout: x is staged in 17 f-windows of 128 frequency rows each
([128(f) partitions, (c, t, r) free], 4 KB descriptors spanning all 128
partitions so all 16 SDMA engines load evenly; issue alternates between the
sync and scalar HWDGE rings).  GPSIMD casts each window to bf16; all matmuls
run in bf16.  Band segments inside a window keep a legal base partition
(0/64) by zero-padding the weight columns outside the band's rows.

Stats: GPSIMD free-dim reduce gives per-partition sums, ACT Square+accum
gives per-partition sum-of-squares; a per-window selection matmul folds the
128 per-f-row partials into per-band (s1, s2).  The per-band (rb, rb*mu)
pair is broadcast to all 128 partitions with a PE transpose + K=1 matmul
(no DRAM round trip).

Sharding: data-parallel over batch B=8, one sample per NeuronCore.
"""

import numpy as np
import ml_dtypes

import concourse.bass as bass
import concourse.tile as tile
from concourse import bacc, mybir

F32 = mybir.dt.float32
BF16 = mybir.dt.bfloat16
AFT = mybir.ActivationFunctionType
ALU = mybir.AluOpType

# ---------------------------------------------------------------- problem dims
WIDTHS = [25] * 10 + [50] * 12 + [100] * 8 + [399]
NB = len(WIDTHS)              # 31
C_IN = 2
T = 512
OUT_CH = 128
EPS = 1e-5
F_TOT = 2049
N_CORES = 8

STARTS = np.concatenate([[0], np.cumsum(WIDTHS)]).astype(int)
CHOFF = np.concatenate([[0], np.cumsum([4 * w for w in WIDTHS])]).astype(int)

NW = 17                       # 16 windows of 128 f-rows + 1 leftover row


def _wrange(w):
    return (128 * w, 2049 if w == 16 else 128 * (w + 1))


# groups: (b0, b1, w_first, w_last_excl) — bands b0..b1-1 live in those windows
GROUPS = [
    (0, 10, 0, 2),
    (10, 17, 1, 5),
    (17, 22, 4, 7),
    (22, 27, 6, 11),
    (27, 31, 10, 17),
]

# weight DMA pieces by window range
WPIECES = [(0, 2), (2, 6), (6, 11), (11, 17)]


def _build_tables():
    # pieces: band cut by window -> one (w, b) entry with partition range and
    # the (k0, k1) contraction class (0/64-based so matmul bases stay legal)
    pieces = []
    for w in range(NW):
        f0, f1 = _wrange(w)
        for b in range(NB):
            s, e = int(STARTS[b]), int(STARTS[b + 1])
            lo, hi = max(s, f0), min(e, f1)
            if lo >= hi:
                continue
            p0, p1 = lo - f0, hi - f0
            pieces.append(dict(w=w, b=b, p0=p0, p1=p1, k0=0, k1=128))

    # weight blob columns per DMA piece; half-columns (base 0 / base 64) pack
    # two slots per 128-row column
    slot_col = {}            # (w, b, c, r) -> (piece_idx, col)
    piece_ncols = []
    for pi, (w0, w1) in enumerate(WPIECES):
        col = 0
        for p in pieces:
            if not (w0 <= p["w"] < w1):
                continue
            for c in range(2):
                for r in range(2):
                    slot_col[(p["w"], p["b"], c, r)] = (pi, col)
                    col += 1
        piece_ncols.append(col)

    # selection-matmul list: one per (window, group) pair
    sel_list = []
    for g, (b0, b1, wf, wl) in enumerate(GROUPS):
        for w in range(wf, wl):
            sel_list.append((w, g))
    sel_list.sort()
    sel_idx = {wg: k for k, wg in enumerate(sel_list)}

    # per-band totals for psum start/stop flags
    band_nmm = {}
    for p in pieces:
        band_nmm[p["b"]] = band_nmm.get(p["b"], 0) + 4
    return pieces, slot_col, piece_ncols, sel_list, sel_idx, band_nmm


PIECES, SLOT_COL, PIECE_NCOLS, SEL_LIST, SEL_IDX, BAND_NMM = _build_tables()
NSEL = len(SEL_LIST)


def _band_group(b):
    for g, (b0, b1, _, _) in enumerate(GROUPS):
        if b0 <= b < b1:
            return g
    raise AssertionError


def _pack_params(W, gamma, beta, bb):
    """Host-side parameter prep: bf16 weight blob pieces, sel matrices,
    uvb (u|v) tensor, invn constants."""
    bf16 = ml_dtypes.bfloat16
    Wg = (W * gamma[None, :]).astype(np.float32)
    Wgb = Wg.astype(bf16)
    Wgb32 = Wgb.astype(np.float32)
    WgbT = np.ascontiguousarray(Wgb.T)          # [8196, 128] bf16

    blobs = [np.zeros((128, nc_, 128), bf16) for nc_ in PIECE_NCOLS]
    for p in PIECES:
        w, b = p["w"], p["b"]
        f0, _ = _wrange(w)
        s = int(STARTS[b])
        wb = WIDTHS[b]
        for c in range(2):
            for r in range(2):
                pi, col = SLOT_COL[(w, b, c, r)]
                for q in range(p["p0"], p["p1"]):
                    ch = int(CHOFF[b]) + r * 2 * wb + c * wb + (f0 + q - s)
                    blobs[pi][q, col, :] = WgbT[ch, :]
    wtps = [np.ascontiguousarray(bl.reshape(128, -1)) for bl in blobs]

    sel = np.zeros((128, NSEL, 32), np.float32)
    for k, (w, g) in enumerate(SEL_LIST):
        b0, b1, _, _ = GROUPS[g]
        f0, f1 = _wrange(w)
        for j, b in enumerate(range(b0, b1)):
            s, e = int(STARTS[b]), int(STARTS[b + 1])
            lo, hi = max(s, f0), min(e, f1)
            if lo < hi:
                sel[lo - f0:hi - f0, k, j] = 1.0
    selb = np.ascontiguousarray(sel.astype(bf16).reshape(128, -1))

    uvb = np.zeros((128, 2 * NB), np.float32)
    for i in range(NB):
        a, e = int(CHOFF[i]), int(CHOFF[i + 1])
        uvb[:, i] = Wgb32[:, a:e].sum(axis=1)                 # u
        uvb[:, NB + i] = W[:, a:e] @ beta[a:e] + bb[i]        # v + b_band
    invn = np.zeros((16, len(GROUPS)), np.float32)
    for g, (b0, b1, _, _) in enumerate(GROUPS):
        for j, b in enumerate(range(b0, b1)):
            invn[j, g] = 1.0 / (4 * WIDTHS[b] * T)
    ident = np.eye(16, dtype=np.float32)
    return wtps, selb, uvb, invn, ident


def _build_nc():
    nc = bacc.Bacc("TRN2")

    x_d = nc.dram_tensor("xin", [C_IN, F_TOT, T, 2], F32, kind="ExternalInput")
    wtp_d = [
        nc.dram_tensor(f"wtp{pi}", [128, PIECE_NCOLS[pi] * 128], BF16,
                       kind="ExternalInput")
        for pi in range(len(WPIECES))
    ]
    sel_d = nc.dram_tensor("sel", [128, NSEL * 32], BF16, kind="ExternalInput")
    uvb_d = nc.dram_tensor("uvb", [128, 2 * NB], F32, kind="ExternalInput")
    invn_d = nc.dram_tensor("invn", [16, len(GROUPS)], F32, kind="ExternalInput")
    ident_d = nc.dram_tensor("ident", [16, 16], F32, kind="ExternalInput")
    y_d = nc.dram_tensor("y", [OUT_CH, NB, T], F32, kind="ExternalOutput")
    rpack_d = nc.dram_tensor("rpack_scratch", [NB, 2], F32)

    CS = F_TOT * T * 2        # c stride in elements
    FS = T * 2                # f stride

    with tile.TileContext(nc) as tc:
        with tc.tile_pool(name="persist", bufs=1) as persist, \
             tc.tile_pool(name="stage", bufs=5) as stage, \
             tc.tile_pool(name="xbp", bufs=4) as xbp, \
             tc.tile_pool(name="sqp", bufs=2) as sqp, \
             tc.tile_pool(name="wtpp", bufs=2) as wtpp, \
             tc.tile_pool(name="osbp", bufs=2) as osbp, \
             tc.tile_pool(name="grp", bufs=2) as grp, \
             tc.tile_pool(name="psacc", bufs=4, space="PSUM") as psacc, \
             tc.tile_pool(name="pss1", bufs=2, space="PSUM") as pss1, \
             tc.tile_pool(name="psstat", bufs=2, space="PSUM") as psstat:

            # ---------------- constants / parameter loads -------------------
            selt = persist.tile([128, NSEL, 32], BF16)
            nc.scalar.dma_start(out=selt, in_=sel_d[:])
            uvbt = persist.tile([128, 2 * NB], F32)
            nc.scalar.dma_start(out=uvbt, in_=uvb_d[:])
            invnt = persist.tile([16, len(GROUPS)], F32)
            nc.scalar.dma_start(out=invnt, in_=invn_d[:])

            ones = persist.tile([1, 128], F32)
            nc.vector.memset(ones, 1.0)
            idt = persist.tile([16, 16], F32)
            nc.scalar.dma_start(out=idt, in_=ident_d[:])
            epst = persist.tile([16, 1], F32)
            nc.vector.memset(epst, EPS)
            strip = persist.tile([128, NW, 2], F32)
            nc.vector.memset(strip, 0.0)
            stripb = persist.tile([128, NW, 2], BF16)

            wts = []
            for pi in range(len(WPIECES)):
                wt = wtpp.tile([128, PIECE_NCOLS[pi], 128], BF16, tag="wtpc",
                               name=f"wtp{pi}")
                nc.scalar.dma_start(out=wt[:, :, :].rearrange("p a b -> p (a b)"),
                                    in_=wtp_d[pi][:])
                wts.append(wt)

            # ---------------- streaming over windows ------------------------
            win_pieces = {}
            for p in PIECES:
                win_pieces.setdefault(p["w"], []).append(p)
            band_done = {}
            band_psum = {}
            osb_tiles = {}
            group_of_w_last = {}   # group -> last window
            stats_ps = {}
            s1_ps = {}
            s1_done = {g: 0 for g in range(len(GROUPS))}
            sel_done = {g: 0 for g in range(len(GROUPS))}
            sel_total = {g: sum(1 for (w_, g_) in SEL_LIST if g_ == g)
                         for g in range(len(GROUPS))}

            xr = x_d
            base = xr[0, 0, 0, 0]

            for w in range(NW):
                f0, f1 = _wrange(w)
                nf = f1 - f0
                xt = stage.tile([128, 2048], F32, tag="xt", name=f"xt{w}")
                src = bass.AP(
                    tensor=base.tensor,
                    offset=base.offset + f0 * FS,
                    ap=[[FS, nf], [CS, 2], [1, 1024]])
                eng = nc.sync if w % 2 == 0 else nc.scalar
                eng.dma_start(
                    out=xt[0:nf, :].rearrange("p (c j) -> p c j", c=2), in_=src)

                # bf16 cast (GPSIMD) + per-partition stats
                xb = xbp.tile([128, 2048], BF16, tag="xb", name=f"xb{w}")
                if nf < 128:
                    nc.vector.memset(xb[:, :], 0.0)
                nc.gpsimd.tensor_copy(out=xb[0:nf, :], in_=xt[0:nf, :])
                sq = sqp.tile([128, 2048], BF16, tag="sq", name=f"sq{w}")
                nc.scalar.activation(out=sq[0:nf, :], in_=xb[0:nf, :],
                                     func=AFT.Square,
                                     accum_out=strip[0:nf, w, 0:1])

                # main matmuls
                for p in win_pieces[w]:
                    b = p["b"]
                    g = _band_group(b)
                    if b not in band_psum:
                        band_psum[b] = psacc.tile([128, T], F32, tag="acc",
                                                  name=f"acc{b}")
                        band_done[b] = 0
                    k0, k1 = p["k0"], p["k1"]
                    for c in range(2):
                        xv = xb[:, c * 1024:(c + 1) * 1024].rearrange(
                            "p (t r) -> p t r", r=2)
                        for r in range(2):
                            pi, col = SLOT_COL[(w, b, c, r)]
                            band_done[b] += 1
                            nc.tensor.matmul(
                                band_psum[b][:],
                                wts[pi][k0:k1, col, :],
                                xv[k0:k1, :, r],
                                start=(band_done[b] == 1),
                                stop=(band_done[b] == BAND_NMM[b]),
                            )
                    if band_done[b] == BAND_NMM[b]:
                        b0, b1, _, _ = GROUPS[g]
                        if g not in osb_tiles:
                            osb_tiles[g] = osbp.tile([128, 10, T], F32,
                                                     tag="osb", name=f"osb{g}")
                        acc = band_psum.pop(b)
                        nc.vector.tensor_copy(out=osb_tiles[g][:, b - b0, :],
                                              in_=acc[:])

                # selection matmuls: per-band s1 (direct, N=512) and s2
                # (via the strip of per-partition square-sums, N=2)
                nc.vector.tensor_copy(out=stripb[:, w, :], in_=strip[:, w, :])
                for g in range(len(GROUPS)):
                    if (w, g) not in SEL_IDX:
                        continue
                    k = SEL_IDX[(w, g)]
                    if g not in stats_ps:
                        stats_ps[g] = psstat.tile([32, 2], F32, tag="stat",
                                                  name=f"stat{g}")
                        s1_ps[g] = pss1.tile([32, T], F32, tag="s1",
                                             name=f"s1{g}")
                    sel_done[g] += 1
                    nc.tensor.matmul(
                        stats_ps[g][:],
                        selt[:, k, :],
                        stripb[:, w, :],
                        start=(sel_done[g] == 1),
                        stop=(sel_done[g] == sel_total[g]),
                    )
                    for c in range(2):
                        for h in range(2):
                            s1_done[g] += 1
                            nc.tensor.matmul(
                                s1_ps[g][:],
                                selt[:, k, :],
                                xb[:, c * 1024 + h * T:
                                   c * 1024 + (h + 1) * T],
                                start=(s1_done[g] == 1),
                                stop=(s1_done[g] == 4 * sel_total[g]),
                            )

                    if sel_done[g] == sel_total[g]:
                        # ---- group stats chain + finalize + store ----
                        b0, b1, _, _ = GROUPS[g]
                        ng = b1 - b0
                        sp = stats_ps.pop(g)
                        s1p = s1_ps.pop(g)
                        s1red = grp.tile([16, 1], F32, tag="s1red",
                                         name=f"s1r{g}")
                        nc.vector.tensor_reduce(out=s1red[0:ng],
                                                in_=s1p[0:ng, :],
                                                axis=mybir.AxisListType.X,
                                                op=ALU.add)
                        mu = grp.tile([16, 1], F32, tag="mu", name=f"mu{g}")
                        nc.vector.tensor_mul(out=mu[0:ng], in0=s1red[0:ng],
                                             in1=invnt[0:ng, g:g + 1])
                        ex2 = grp.tile([16, 1], F32, tag="ex2", name=f"ex2{g}")
                        nc.vector.tensor_mul(out=ex2[0:ng], in0=sp[0:ng, 0:1],
                                             in1=invnt[0:ng, g:g + 1])
                        musq = grp.tile([16, 1], F32, tag="musq",
                                        name=f"musq{g}")
                        nc.vector.tensor_mul(out=musq[0:ng], in0=mu[0:ng],
                                             in1=mu[0:ng])
                        var = grp.tile([16, 1], F32, tag="var", name=f"var{g}")
                        nc.vector.tensor_tensor(out=var[0:ng], in0=ex2[0:ng],
                                                in1=musq[0:ng],
                                                op=ALU.subtract)
                        std = grp.tile([16, 1], F32, tag="std", name=f"std{g}")
                        nc.scalar.activation(out=std[0:ng], in_=var[0:ng],
                                             func=AFT.Sqrt,
                                             bias=epst[0:ng, 0:1])
                        rpack = grp.tile([16, 2], F32, tag="rpack",
                                         name=f"rp{g}")
                        nc.vector.reciprocal(out=rpack[0:ng, 0:1],
                                             in_=std[0:ng])
                        nc.vector.tensor_mul(out=rpack[0:ng, 1:2],
                                             in0=rpack[0:ng, 0:1],
                                             in1=mu[0:ng])

                        # broadcast (rb, rb*mu) to all 128 partitions via
                        # a DRAM round trip (partition-flattening)
                        nc.sync.dma_start(out=rpack_d[b0:b1, :],
                                          in_=rpack[0:ng, :])
                        rbu = grp.tile([128, 16, 2], F32, tag="rbu",
                                       name=f"rbu{g}")
                        src_r = rpack_d[b0:b0 + 1, 0:1]
                        nc.sync.dma_start(
                            out=rbu[:, 0:ng, :],
                            in_=bass.AP(tensor=src_r.tensor,
                                        offset=src_r.offset,
                                        ap=[[0, 128], [2, ng], [1, 2]]),
                        )

                        tru = grp.tile([128, 16], F32, tag="tru",
                                       name=f"tru{g}")
                        nc.vector.tensor_mul(out=tru[:, 0:ng],
                                             in0=rbu[:, 0:ng, 1],
                                             in1=uvbt[:, b0:b0 + ng])
                        bbv = grp.tile([128, 16], F32, tag="bbv",
                                       name=f"bbv{g}")
                        nc.vector.tensor_tensor(
                            out=bbv[:, 0:ng],
                            in0=uvbt[:, NB + b0:NB + b0 + ng],
                            in1=tru[:, 0:ng], op=ALU.subtract)

                        osb = osb_tiles.pop(g)
                        for j in range(ng):
                            nc.vector.tensor_scalar(
                                out=osb[:, j, :], in0=osb[:, j, :],
                                scalar1=rbu[:, j, 0:1],
                                scalar2=bbv[:, j:j + 1],
                                op0=ALU.mult, op1=ALU.add,
                            )
                        nc.sync.dma_start(out=y_d[:, b0:b1, :],
                                          in_=osb[:, 0:ng, :])

    nc.finalize()
    return nc


_NC_CACHE = None


def _get_nc():
    global _NC_CACHE
    if _NC_CACHE is None:
        _NC_CACHE = _build_nc()
    return _NC_CACHE


def kernel(x, gamma, beta, W, b):
    from concourse.bass_utils import run_bass_kernel_spmd

    x = np.asarray(x, dtype=np.float32)
    gamma = np.asarray(gamma, dtype=np.float32)
    beta = np.asarray(beta, dtype=np.float32)
    W = np.asarray(W, dtype=np.float32)
    b = np.asarray(b, dtype=np.float32)

    wtps, selb, uvb, invn, ident = _pack_params(W, gamma, beta, b)
    nc = _get_nc()
    shared = {f"wtp{pi}": wtps[pi] for pi in range(len(wtps))}
    shared.update({"sel": selb, "uvb": uvb, "invn": invn, "ident": ident})
    in_maps = [
        dict(shared, xin=np.ascontiguousarray(x[i]))
        for i in range(N_CORES)
    ]
    res = run_bass_kernel_spmd(nc, in_maps, list(range(N_CORES)))
    return np.stack([res.results[i]["y"] for i in range(N_CORES)], axis=0)


# revision 10
# speedup vs baseline: 1.5974x; 1.5974x over previous
"""BandSplit (BSRNN-style) Trainium2 kernel — f-window streaming design.

Folded form (r_b = rsqrt(var+eps); mu, r_b per band+sample):
  y = r_b * (Wg @ h) + (v + b_band - r_b*mu*u)
  Wg = W * gamma (per column), u = sum_cols(Wg_bf16), v = W @ beta + b_band
so the big matmul runs on raw h and GroupNorm is a per-band scalar scale plus
a per-output-channel bias on the matmul output.

Data layout: x is staged in 17 f-windows of 128 frequency rows each
([128(f) partitions, (c, t, r) free], 4 KB descriptors spanning all 128
partitions so all 16 SDMA engines load evenly; issue alternates between the
sync and scalar HWDGE rings).  GPSIMD casts each window to bf16; all matmuls
run in bf16.  Band segments inside a window keep a legal base partition
(0/64) by zero-padding the weight columns outside the band's rows.

Stats: GPSIMD free-dim reduce gives per-partition sums, ACT Square+accum
gives per-partition sum-of-squares; a per-window selection matmul folds the
128 per-f-row partials into per-band (s1, s2).  The per-band (rb, rb*mu)
pair is broadcast to all 128 partitions with a PE transpose + K=1 matmul
(no DRAM round trip).

Sharding: data-parallel over batch B=8, one sample per NeuronCore.
"""

import numpy as np
import ml_dtypes

import concourse.bass as bass
import concourse.tile as tile
from concourse import bacc, mybir

F32 = mybir.dt.float32
BF16 = mybir.dt.bfloat16
AFT = mybir.ActivationFunctionType
ALU = mybir.AluOpType

# ---------------------------------------------------------------- problem dims
WIDTHS = [25] * 10 + [50] * 12 + [100] * 8 + [399]
NB = len(WIDTHS)              # 31
C_IN = 2
T = 512
OUT_CH = 128
EPS = 1e-5
F_TOT = 2049
N_CORES = 8

STARTS = np.concatenate([[0], np.cumsum(WIDTHS)]).astype(int)
CHOFF = np.concatenate([[0], np.cumsum([4 * w for w in WIDTHS])]).astype(int)

NW = 17                       # 16 windows of 128 f-rows + 1 leftover row


def _wrange(w):
    return (128 * w, 2049 if w == 16 else 128 * (w + 1))


# groups: (b0, b1, w_first, w_last_excl) — bands b0..b1-1 live in those windows
GROUPS = [
    (0, 10, 0, 2),
    (10, 17, 1, 5),
    (17, 22, 4, 7),
    (22, 27, 6, 11),
    (27, 31, 10, 17),
]

# weight DMA pieces by window range
WPIECES = [(0, 2), (2, 6), (6, 11), (11, 17)]


def _build_tables():
    # pieces: band cut by window -> one (w, b) entry with partition range and
    # the (k0, k1) contraction class (0/64-based so matmul bases stay legal)
    pieces = []
    for w in range(NW):
        f0, f1 = _wrange(w)
        for b in range(NB):
            s, e = int(STARTS[b]), int(STARTS[b + 1])
            lo, hi = max(s, f0), min(e, f1)
            if lo >= hi:
                continue
            p0, p1 = lo - f0, hi - f0
            pieces.append(dict(w=w, b=b, p0=p0, p1=p1, k0=0, k1=128))

    # weight blob columns per DMA piece; half-columns (base 0 / base 64) pack
    # two slots per 128-row column
    slot_col = {}            # (w, b, c, r) -> (piece_idx, col)
    piece_ncols = []
    for pi, (w0, w1) in enumerate(WPIECES):
        col = 0
        for p in pieces:
            if not (w0 <= p["w"] < w1):
                continue
            for c in range(2):
                for r in range(2):
                    slot_col[(p["w"], p["b"], c, r)] = (pi, col)
                    col += 1
        piece_ncols.append(col)

    # selection-matmul list: one per (window, group) pair
    sel_list = []
    for g, (b0, b1, wf, wl) in enumerate(GROUPS):
        for w in range(wf, wl):
            sel_list.append((w, g))
    sel_list.sort()
    sel_idx = {wg: k for k, wg in enumerate(sel_list)}

    # per-band totals for psum start/stop flags
    band_nmm = {}
    for p in pieces:
        band_nmm[p["b"]] = band_nmm.get(p["b"], 0) + 4
    return pieces, slot_col, piece_ncols, sel_list, sel_idx, band_nmm


PIECES, SLOT_COL, PIECE_NCOLS, SEL_LIST, SEL_IDX, BAND_NMM = _build_tables()
NSEL = len(SEL_LIST)


def _band_group(b):
    for g, (b0, b1, _, _) in enumerate(GROUPS):
        if b0 <= b < b1:
            return g
    raise AssertionError


def _pack_params(W, gamma, beta, bb):
    """Host-side parameter prep: bf16 weight blob pieces, sel matrices,
    uvb (u|v) tensor, invn constants."""
    bf16 = ml_dtypes.bfloat16
    Wg = (W * gamma[None, :]).astype(np.float32)
    Wgb = Wg.astype(bf16)
    Wgb32 = Wgb.astype(np.float32)
    WgbT = np.ascontiguousarray(Wgb.T)          # [8196, 128] bf16

    blobs = [np.zeros((128, nc_, 128), bf16) for nc_ in PIECE_NCOLS]
    for p in PIECES:
        w, b = p["w"], p["b"]
        f0, _ = _wrange(w)
        s = int(STARTS[b])
        wb = WIDTHS[b]
        for c in range(2):
            for r in range(2):
                pi, col = SLOT_COL[(w, b, c, r)]
                for q in range(p["p0"], p["p1"]):
                    ch = int(CHOFF[b]) + r * 2 * wb + c * wb + (f0 + q - s)
                    blobs[pi][q, col, :] = WgbT[ch, :]
    wtps = [np.ascontiguousarray(bl.reshape(128, -1)) for bl in blobs]

    sel = np.zeros((128, NSEL, 32), np.float32)
    for k, (w, g) in enumerate(SEL_LIST):
        b0, b1, _, _ = GROUPS[g]
        f0, f1 = _wrange(w)
        for j, b in enumerate(range(b0, b1)):
            s, e = int(STARTS[b]), int(STARTS[b + 1])
            lo, hi = max(s, f0), min(e, f1)
            if lo < hi:
                sel[lo - f0:hi - f0, k, j] = 1.0
    selb = np.ascontiguousarray(sel.astype(bf16).reshape(128, -1))

    uvb = np.zeros((128, 2 * NB), np.float32)
    for i in range(NB):
        a, e = int(CHOFF[i]), int(CHOFF[i + 1])
        uvb[:, i] = Wgb32[:, a:e].sum(axis=1)                 # u
        uvb[:, NB + i] = W[:, a:e] @ beta[a:e] + bb[i]        # v + b_band
    invn = np.zeros((16, len(GROUPS)), np.float32)
    for g, (b0, b1, _, _) in enumerate(GROUPS):
        for j, b in enumerate(range(b0, b1)):
            invn[j, g] = 1.0 / (4 * WIDTHS[b] * T)
    ident = np.eye(16, dtype=np.float32)
    return wtps, selb, uvb, invn, ident


def _build_nc():
    nc = bacc.Bacc("TRN2")

    x_d = nc.dram_tensor("xin", [C_IN, F_TOT, T, 2], F32, kind="ExternalInput")
    wtp_d = [
        nc.dram_tensor(f"wtp{pi}", [128, PIECE_NCOLS[pi] * 128], BF16,
                       kind="ExternalInput")
        for pi in range(len(WPIECES))
    ]
    sel_d = nc.dram_tensor("sel", [128, NSEL * 32], BF16, kind="ExternalInput")
    uvb_d = nc.dram_tensor("uvb", [128, 2 * NB], F32, kind="ExternalInput")
    invn_d = nc.dram_tensor("invn", [16, len(GROUPS)], F32, kind="ExternalInput")
    ident_d = nc.dram_tensor("ident", [16, 16], F32, kind="ExternalInput")
    y_d = nc.dram_tensor("y", [OUT_CH, NB, T], F32, kind="ExternalOutput")
    rpack_d = nc.dram_tensor("rpack_scratch", [NB, 2], F32)

    CS = F_TOT * T * 2        # c stride in elements
    FS = T * 2                # f stride

    with tile.TileContext(nc) as tc:
        with tc.tile_pool(name="persist", bufs=1) as persist, \
             tc.tile_pool(name="xbp", bufs=6) as xbp, \
             tc.tile_pool(name="sqp", bufs=2) as sqp, \
             tc.tile_pool(name="wtpp", bufs=2) as wtpp, \
             tc.tile_pool(name="osbp", bufs=2) as osbp, \
             tc.tile_pool(name="grp", bufs=2) as grp, \
             tc.tile_pool(name="psacc", bufs=4, space="PSUM") as psacc, \
             tc.tile_pool(name="pss1", bufs=2, space="PSUM") as pss1, \
             tc.tile_pool(name="psstat", bufs=2, space="PSUM") as psstat:

            # ---------------- constants / parameter loads -------------------
            selt = persist.tile([128, NSEL, 32], BF16)
            nc.scalar.dma_start(out=selt, in_=sel_d[:])
            uvbt = persist.tile([128, 2 * NB], F32)
            nc.scalar.dma_start(out=uvbt, in_=uvb_d[:])
            invnt = persist.tile([16, len(GROUPS)], F32)
            nc.scalar.dma_start(out=invnt, in_=invn_d[:])

            ones = persist.tile([1, 128], F32)
            nc.vector.memset(ones, 1.0)
            idt = persist.tile([16, 16], F32)
            nc.scalar.dma_start(out=idt, in_=ident_d[:])
            epst = persist.tile([16, 1], F32)
            nc.vector.memset(epst, EPS)
            strip = persist.tile([128, NW, 2], F32)
            nc.vector.memset(strip, 0.0)
            stripb = persist.tile([128, NW, 2], BF16)

            wts = []
            for pi in range(len(WPIECES)):
                wt = wtpp.tile([128, PIECE_NCOLS[pi], 128], BF16, tag="wtpc",
                               name=f"wtp{pi}")
                nc.scalar.dma_start(out=wt[:, :, :].rearrange("p a b -> p (a b)"),
                                    in_=wtp_d[pi][:])
                wts.append(wt)

            # ---------------- streaming over windows ------------------------
            win_pieces = {}
            for p in PIECES:
                win_pieces.setdefault(p["w"], []).append(p)
            band_done = {}
            band_psum = {}
            osb_tiles = {}
            group_of_w_last = {}   # group -> last window
            stats_ps = {}
            s1_ps = {}
            s1_done = {g: 0 for g in range(len(GROUPS))}
            sel_done = {g: 0 for g in range(len(GROUPS))}
            sel_total = {g: sum(1 for (w_, g_) in SEL_LIST if g_ == g)
                         for g in range(len(GROUPS))}

            xr = x_d
            base = xr[0, 0, 0, 0]

            for w in range(NW):
                f0, f1 = _wrange(w)
                nf = f1 - f0
                # stage + f32->bf16 cast in one SWDGE DMA (cast happens in
                # the DMA engines, round-to-nearest-even)
                xb = xbp.tile([128, 2048], BF16, tag="xb", name=f"xb{w}")
                if nf < 128:
                    nc.vector.memset(xb[:, :], 0.0)
                src = bass.AP(
                    tensor=base.tensor,
                    offset=base.offset + f0 * FS,
                    ap=[[FS, nf], [CS, 2], [1, 1024]])
                nc.gpsimd.dma_start(
                    out=xb[0:nf, :].rearrange("p (c j) -> p c j", c=2), in_=src)
                sq = sqp.tile([128, 2048], BF16, tag="sq", name=f"sq{w}")
                nc.scalar.activation(out=sq[0:nf, :], in_=xb[0:nf, :],
                                     func=AFT.Square,
                                     accum_out=strip[0:nf, w, 0:1])

                # main matmuls
                for p in win_pieces[w]:
                    b = p["b"]
                    g = _band_group(b)
                    if b not in band_psum:
                        band_psum[b] = psacc.tile([128, T], F32, tag="acc",
                                                  name=f"acc{b}")
                        band_done[b] = 0
                    k0, k1 = p["k0"], p["k1"]
                    for c in range(2):
                        xv = xb[:, c * 1024:(c + 1) * 1024].rearrange(
                            "p (t r) -> p t r", r=2)
                        for r in range(2):
                            pi, col = SLOT_COL[(w, b, c, r)]
                            band_done[b] += 1
                            nc.tensor.matmul(
                                band_psum[b][:],
                                wts[pi][k0:k1, col, :],
                                xv[k0:k1, :, r],
                                start=(band_done[b] == 1),
                                stop=(band_done[b] == BAND_NMM[b]),
                            )
                    if band_done[b] == BAND_NMM[b]:
                        b0, b1, _, _ = GROUPS[g]
                        if g not in osb_tiles:
                            osb_tiles[g] = osbp.tile([128, 10, T], F32,
                                                     tag="osb", name=f"osb{g}")
                        acc = band_psum.pop(b)
                        nc.vector.tensor_copy(out=osb_tiles[g][:, b - b0, :],
                                              in_=acc[:])

                # selection matmuls: per-band s1 (direct, N=512) and s2
                # (via the strip of per-partition square-sums, N=2)
                nc.vector.tensor_copy(out=stripb[:, w, :], in_=strip[:, w, :])
                for g in range(len(GROUPS)):
                    if (w, g) not in SEL_IDX:
                        continue
                    k = SEL_IDX[(w, g)]
                    if g not in stats_ps:
                        stats_ps[g] = psstat.tile([32, 2], F32, tag="stat",
                                                  name=f"stat{g}")
                        s1_ps[g] = pss1.tile([32, T], F32, tag="s1",
                                             name=f"s1{g}")
                    sel_done[g] += 1
                    nc.tensor.matmul(
                        stats_ps[g][:],
                        selt[:, k, :],
                        stripb[:, w, :],
                        start=(sel_done[g] == 1),
                        stop=(sel_done[g] == sel_total[g]),
                    )
                    for c in range(2):
                        for h in range(2):
                            s1_done[g] += 1
                            nc.tensor.matmul(
                                s1_ps[g][:],
                                selt[:, k, :],
                                xb[:, c * 1024 + h * T:
                                   c * 1024 + (h + 1) * T],
                                start=(s1_done[g] == 1),
                                stop=(s1_done[g] == 4 * sel_total[g]),
                            )

                    if sel_done[g] == sel_total[g]:
                        # ---- group stats chain + finalize + store ----
                        b0, b1, _, _ = GROUPS[g]
                        ng = b1 - b0
                        sp = stats_ps.pop(g)
                        s1p = s1_ps.pop(g)
                        s1red = grp.tile([16, 1], F32, tag="s1red",
                                         name=f"s1r{g}")
                        nc.vector.tensor_reduce(out=s1red[0:ng],
                                                in_=s1p[0:ng, :],
                                                axis=mybir.AxisListType.X,
                                                op=ALU.add)
                        mu = grp.tile([16, 1], F32, tag="mu", name=f"mu{g}")
                        nc.vector.tensor_mul(out=mu[0:ng], in0=s1red[0:ng],
                                             in1=invnt[0:ng, g:g + 1])
                        ex2 = grp.tile([16, 1], F32, tag="ex2", name=f"ex2{g}")
                        nc.vector.tensor_mul(out=ex2[0:ng], in0=sp[0:ng, 0:1],
                                             in1=invnt[0:ng, g:g + 1])
                        musq = grp.tile([16, 1], F32, tag="musq",
                                        name=f"musq{g}")
                        nc.vector.tensor_mul(out=musq[0:ng], in0=mu[0:ng],
                                             in1=mu[0:ng])
                        var = grp.tile([16, 1], F32, tag="var", name=f"var{g}")
                        nc.vector.tensor_tensor(out=var[0:ng], in0=ex2[0:ng],
                                                in1=musq[0:ng],
                                                op=ALU.subtract)
                        std = grp.tile([16, 1], F32, tag="std", name=f"std{g}")
                        nc.scalar.activation(out=std[0:ng], in_=var[0:ng],
                                             func=AFT.Sqrt,
                                             bias=epst[0:ng, 0:1])
                        rpack = grp.tile([16, 2], F32, tag="rpack",
                                         name=f"rp{g}")
                        nc.vector.reciprocal(out=rpack[0:ng, 0:1],
                                             in_=std[0:ng])
                        nc.vector.tensor_mul(out=rpack[0:ng, 1:2],
                                             in0=rpack[0:ng, 0:1],
                                             in1=mu[0:ng])

                        # broadcast (rb, rb*mu) to all 128 partitions via
                        # a DRAM round trip (partition-flattening)
                        nc.sync.dma_start(out=rpack_d[b0:b1, :],
                                          in_=rpack[0:ng, :])
                        rbu = grp.tile([128, 16, 2], F32, tag="rbu",
                                       name=f"rbu{g}")
                        src_r = rpack_d[b0:b0 + 1, 0:1]
                        nc.sync.dma_start(
                            out=rbu[:, 0:ng, :],
                            in_=bass.AP(tensor=src_r.tensor,
                                        offset=src_r.offset,
                                        ap=[[0, 128], [2, ng], [1, 2]]),
                        )

                        tru = grp.tile([128, 16], F32, tag="tru",
                                       name=f"tru{g}")
                        nc.vector.tensor_mul(out=tru[:, 0:ng],
                                             in0=rbu[:, 0:ng, 1],
                                             in1=uvbt[:, b0:b0 + ng])
                        bbv = grp.tile([128, 16], F32, tag="bbv",
                                       name=f"bbv{g}")
                        nc.vector.tensor_tensor(
                            out=bbv[:, 0:ng],
                            in0=uvbt[:, NB + b0:NB + b0 + ng],
                            in1=tru[:, 0:ng], op=ALU.subtract)

                        osb = osb_tiles.pop(g)
                        for j in range(ng):
                            nc.vector.tensor_scalar(
                                out=osb[:, j, :], in0=osb[:, j, :],
                                scalar1=rbu[:, j, 0:1],
                                scalar2=bbv[:, j:j + 1],
                                op0=ALU.mult, op1=ALU.add,
                            )
                        nc.sync.dma_start(out=y_d[:, b0:b1, :],
                                          in_=osb[:, 0:ng, :])

    nc.finalize()
    return nc


_NC_CACHE = None


def _get_nc():
    global _NC_CACHE
    if _NC_CACHE is None:
        _NC_CACHE = _build_nc()
    return _NC_CACHE


def kernel(x, gamma, beta, W, b):
    from concourse.bass_utils import run_bass_kernel_spmd

    x = np.asarray(x, dtype=np.float32)
    gamma = np.asarray(gamma, dtype=np.float32)
    beta = np.asarray(beta, dtype=np.float32)
    W = np.asarray(W, dtype=np.float32)
    b = np.asarray(b, dtype=np.float32)

    wtps, selb, uvb, invn, ident = _pack_params(W, gamma, beta, b)
    nc = _get_nc()
    shared = {f"wtp{pi}": wtps[pi] for pi in range(len(wtps))}
    shared.update({"sel": selb, "uvb": uvb, "invn": invn, "ident": ident})
    in_maps = [
        dict(shared, xin=np.ascontiguousarray(x[i]))
        for i in range(N_CORES)
    ]
    res = run_bass_kernel_spmd(nc, in_maps, list(range(N_CORES)))
    return np.stack([res.results[i]["y"] for i in range(N_CORES)], axis=0)
